# revision 3
# baseline (speedup 1.0000x reference)
"""Conv2d 3x3 via 1-D Winograd F(4,3) along the kh (row) axis.

out[4b+i] (i=0..3) from 6 products m_k per block of 4 output rows:
  d = BT @ rows(4b..4b+5), m_k = (G w)_k * d_k summed over ci (PE matmul,
  kw taps accumulated directly in PSUM), out = AT @ m + bias.
6 multiplies per 4x1 outputs instead of 12: a 2x TensorEngine FLOP cut
vs direct conv (1.33x vs the previous F(2,3) kernel).

Per core (4 images): input rows pre-transformed into D_k[128, 14, 58]
(k=0..5, 14 blocks) via fused scalar_tensor_tensor ops on DVE, weights
host-transformed into G_k[ci, kw, co] (bf16). For each (image, co_tile,
chunk of 7 blocks): 6 PSUM banks M_k, each accumulating 6 matmuls
(2 ci-tiles x 3 kw taps, K=128, N=7*56=392). Output transform spread
across ACT (PSUM evacuation of m0/m1+bias/m3/m5, scale ops 2v/4u/8v),
DVE (combos with one PSUM operand each - HW allows only one PSUM read
per 2-input op) and GpSimd (SBUF-side adds; GpSimd has no PSUM port).
Transforms for image n+1 are emitted interleaved between image n's
groups (engine queues are strict FIFO).
"""

import numpy as np
import ml_dtypes

import concourse.bass as bass
import concourse.mybir as mybir
from concourse import bacc
from concourse.tile import TileContext
from concourse.bass_utils import run_bass_kernel_spmd

P = 128
N_CORES = 8
NIMG = 4
CIN = 256
COUT = 256
H = W = 56
WP = 58
HPP = 60                   # 1 pad + 56 + 1 pad + 2 zero rows (for /4 blocks)
CI_T = 2
CO_T = 2
NK = 6                     # Winograd F(4,3) indices
HB = 14                    # output row blocks of 4
CHUNKS = [(0, 7), (7, 7)]  # (first block, n blocks)
KORD = [1, 3, 0, 5, 2, 4]  # ACT-evacuated tiles (1,3,0,5) first

ADD = mybir.AluOpType.add
SUB = mybir.AluOpType.subtract
MULT = mybir.AluOpType.mult
IDENT = mybir.ActivationFunctionType.Identity

_cached = {}


def _build_nc():
    nc = bacc.Bacc("TRN2", target_bir_lowering=False, debug=False,
                   num_devices=N_CORES)

    ip_h = nc.declare_dram_parameter("ip", [NIMG, CIN, HPP, WP],
                                     mybir.dt.bfloat16, isOutput=False)
    w_h = nc.declare_dram_parameter("weight", [P, CO_T * CI_T * NK * 3 * P],
                                    mybir.dt.bfloat16, isOutput=False)
    b_h = nc.declare_dram_parameter("bias", [P, CO_T],
                                    mybir.dt.float32, isOutput=False)
    out_h = nc.declare_dram_parameter("out", [NIMG, COUT, H, W],
                                      mybir.dt.float32, isOutput=True)

    ip_v = ip_h.ap().rearrange("n (t p) h w -> n t p h w", p=P)
    w_v = w_h.ap()
    out_v = out_h.ap().rearrange("n (t p) h w -> n t p (h w)", p=P)

    def woff(ot, it, k, kw):
        return (((ot * CI_T + it) * NK + k) * 3 + kw) * P

    WB = NK * 3 * P  # one (ot, it) weight block: 2304 cols

    with TileContext(nc) as tc:
        with (
            tc.tile_pool(name="const", bufs=1) as cpool,
            tc.tile_pool(name="p0", bufs=4) as p0pool,      # img0 half pads
            tc.tile_pool(name="padf", bufs=4) as pfpool,    # img1-3 full pads
            tc.tile_pool(name="dt", bufs=24) as dpool,      # D_k tiles
            tc.tile_pool(name="tmp", bufs=6) as tpool,      # transform temps
            tc.tile_pool(name="stg", bufs=30) as spool,     # out-transform stage
            tc.tile_pool(name="outs", bufs=4) as opool,
            tc.tile_pool(name="psum", bufs=8, space="PSUM") as pspool,
        ):
            wt = cpool.tile([P, CO_T * CI_T * NK * 3 * P], mybir.dt.bfloat16)
            bt = cpool.tile([P, CO_T], mybir.dt.float32)

            # ---- input DMAs, staged in PE-consumption order ----
            tops = [None] * CI_T
            bots = [None] * CI_T
            nc.sync.dma_start(out=wt[:, 0:WB], in_=w_v[:, 0:WB])  # ot0 it0
            for t in range(CI_T):
                pt = p0pool.tile([P, 32, WP], mybir.dt.bfloat16,
                                 name=f"top_{t}", tag="p0")
                nc.sync.dma_start(out=pt[:], in_=ip_v[0, t, :, 0:32])
                tops[t] = pt
            nc.sync.dma_start(out=wt[:, WB:2 * WB],
                              in_=w_v[:, WB:2 * WB])               # ot0 it1
            nc.sync.dma_start(out=bt[:], in_=b_h.ap())
            for t in range(CI_T):
                pb = p0pool.tile([P, 32, WP], mybir.dt.bfloat16,
                                 name=f"bot_{t}", tag="p0")
                nc.sync.dma_start(out=pb[:], in_=ip_v[0, t, :, 28:60])
                bots[t] = pb
            nc.sync.dma_start(out=wt[:, 2 * WB:], in_=w_v[:, 2 * WB:])  # ot1
            fulls = [[None] * CI_T for _ in range(NIMG)]
            for n in range(1, NIMG):
                for t in range(CI_T):
                    pf = pfpool.tile([P, HPP, WP], mybir.dt.bfloat16,
                                     tag="padfull", name=f"pf_{n}_{t}")
                    nc.sync.dma_start(out=pf[:], in_=ip_v[n, t])
                    fulls[n][t] = pf

            dall = [None] * NIMG

            def _alloc_d(n):
                ds = [[None] * NK for _ in range(CI_T)]
                for t in range(CI_T):
                    for k in range(NK):
                        ds[t][k] = dpool.tile([P, HB, WP], mybir.dt.bfloat16,
                                              tag="dk", name=f"d_{n}_{t}_{k}")
                dall[n] = ds

            # ---- D transform: D_k = BT @ rows, via fused STT ops ----
            # A = -4*r2 + r4 ; B = -4*r1 + r3 ; C = r4 - r2 ; V = r3 - r1
            # D1 = A+B ; D2 = A-B ; D3 = 2V+C ; D4 = -2V+C
            # D0 = 4*(r0-r2) + C ; D5 = -4V + (r5-r3)
            def _transform(n, t, pad, b0, nb, half, eng, state):
                """Emit half (0 or 1) of the D ops for blocks [b0, b0+nb)."""
                ds = dall[n][t]
                pv = pad.rearrange("p (b f) c -> p b f c", f=4)
                r = [pv[:, 0:nb, j] if j < 4 else pv[:, 1:nb + 1, j - 4]
                     for j in range(6)]
                sl = (slice(None), slice(b0, b0 + nb), slice(None))

                def tmp():
                    tl = tpool.tile([P, HB, WP], mybir.dt.bfloat16, tag="tt")
                    return tl[:, 0:nb, :]

                if half == 0:
                    A, B, C, V = tmp(), tmp(), tmp(), tmp()
                    eng.scalar_tensor_tensor(A, r[2], -4.0, r[4], MULT, ADD)
                    eng.scalar_tensor_tensor(B, r[1], -4.0, r[3], MULT, ADD)
                    eng.tensor_tensor(ds[1][sl], A, B, ADD)
                    eng.tensor_tensor(ds[2][sl], A, B, SUB)
                    eng.tensor_tensor(C, r[4], r[2], SUB)
                    eng.tensor_tensor(V, r[3], r[1], SUB)
                    state[(n, t, b0)] = (C, V)
                else:
                    C, V = state.pop((n, t, b0))
                    eng.scalar_tensor_tensor(ds[3][sl], V, 2.0, C, MULT, ADD)
                    eng.scalar_tensor_tensor(ds[4][sl], V, -2.0, C, MULT, ADD)
                    E, G0 = tmp(), tmp()
                    eng.tensor_tensor(E, r[0], r[2], SUB)
                    eng.scalar_tensor_tensor(ds[0][sl], E, 4.0, C, MULT, ADD)
                    eng.tensor_tensor(G0, r[5], r[3], SUB)
                    eng.scalar_tensor_tensor(ds[5][sl], V, -4.0, G0, MULT, ADD)

            tstate = {}

            # ---- matmul + output transform per (n, ot, chunk) ----
            def _group(n, ot, b0, nb, it_outer):
                N = nb * W
                ds = dall[n]
                ms = {}
                for k in KORD:
                    ms[k] = pspool.tile([P, N], mybir.dt.float32,
                                        name=f"m_{n}_{ot}_{b0}_{k}",
                                        tag="mpsum")
                if it_outer:
                    order = [(it, k, kw) for it in range(CI_T)
                             for k in KORD for kw in range(3)]
                else:
                    order = [(it, k, kw) for k in KORD
                             for it in range(CI_T) for kw in range(3)]
                for (it, k, kw) in order:
                    rhs = ds[it][k][:, b0:b0 + nb, kw:kw + W]
                    o = woff(ot, it, k, kw)
                    nc.tensor.matmul(
                        ms[k][:], wt[:, o:o + P], rhs,
                        start=(it == 0 and kw == 0),
                        stop=(it == CI_T - 1 and kw == 2),
                    )

                # output transform:
                # o0 = m0 + s + u        s = m1 + m2 (+bias)
                # o1 = d + 2v            d = m1 - m2 (+bias)
                # o2 = s + 4u            u = m3 + m4
                # o3 = d + 8v + m5       v = m3 - m4
                bias = bt[:, ot:ot + 1]

                def stg(nm):
                    return spool.tile([P, N], mybir.dt.float32,
                                      name=f"{nm}_{n}_{ot}_{b0}", tag="stg")

                m1b, m3c, m0c, m5c = stg("m1b"), stg("m3c"), stg("m0c"), stg("m5c")
                s_b, d_b, u, v = stg("s"), stg("d"), stg("u"), stg("v")
                t1, t2, t3 = stg("t1"), stg("t2"), stg("t3")
                sm, w3 = stg("sm"), stg("w3")
                ob = opool.tile([P, nb, 4, W], mybir.dt.float32,
                                name=f"ob_{n}_{ot}_{b0}", tag="ob")
                V_ = nc.vector
                A_ = nc.scalar
                G_ = nc.gpsimd
                A_.activation(m1b[:], ms[1][:], IDENT, bias=bias)
                A_.activation(m3c[:], ms[3][:], IDENT)
                A_.activation(m0c[:], ms[0][:], IDENT)
                A_.activation(m5c[:], ms[5][:], IDENT)
                V_.tensor_tensor(s_b[:], m1b[:], ms[2][:], ADD)
                V_.tensor_tensor(d_b[:], m1b[:], ms[2][:], SUB)
                V_.tensor_tensor(u[:], m3c[:], ms[4][:], ADD)
                V_.tensor_tensor(v[:], m3c[:], ms[4][:], SUB)
                A_.activation(t1[:], v[:], IDENT, scale=2.0)
                A_.activation(t2[:], u[:], IDENT, scale=4.0)
                A_.activation(t3[:], v[:], IDENT, scale=8.0)
                G_.tensor_tensor(sm[:], s_b[:], m0c[:], ADD)
                G_.tensor_tensor(ob[:, :, 0, :], sm[:], u[:], ADD)
                G_.tensor_tensor(ob[:, :, 1, :], t1[:], d_b[:], ADD)
                G_.tensor_tensor(ob[:, :, 2, :], t2[:], s_b[:], ADD)
                G_.tensor_tensor(w3[:], t3[:], d_b[:], ADD)
                G_.tensor_tensor(ob[:, :, 3, :], w3[:], m5c[:], ADD)
                nc.sync.dma_start(
                    out=out_v[n, ot, :, 4 * b0 * W:(4 * b0 + 4 * nb) * W],
                    in_=ob[:])

            # ---- schedule ----
            # image 0: transforms first (it-outer groups tolerate late it1)
            for n in range(NIMG):
                _alloc_d(n)
            for t in range(CI_T):
                for half in range(2):
                    _transform(0, t, tops[t], 0, 7, half, nc.vector, tstate)
            for t in range(CI_T):
                for half in range(2):
                    _transform(0, t, bots[t], 7, 7, half, nc.vector, tstate)

            # transform slices for image n+1, interleaved between image n's
            # 4 groups: (it, src-half, op-half)
            def slices_for(n):
                if n >= NIMG:
                    return [None] * 4
                out = []
                for t in range(CI_T):
                    for half in range(2):
                        out.append((n, t, fulls[n][t], 0, HB, half))
                return out

            for n in range(NIMG):
                slices = slices_for(n + 1)
                gi = 0
                for ot in range(CO_T):
                    for (b0, nb) in CHUNKS:
                        it_outer = (n == 0 and ot == 0)
                        _group(n, ot, b0, nb, it_outer)
                        if slices[gi] is not None:
                            sn, st, spad, sb0, snb, shalf = slices[gi]
                            _transform(sn, st, spad, sb0, snb, shalf,
                                       nc.vector, tstate)
                        gi += 1
    nc.finalize()
    return nc


def _prep_inputs(ip, weight, bias):
    bf16 = ml_dtypes.bfloat16
    ipp = np.zeros((ip.shape[0], CIN, HPP, WP), dtype=bf16)
    ipp[:, :, 1:57, 1:57] = ip.astype(bf16)
    # Winograd F(4,3) weight transform along kh: G_k[ci, kw, co]
    G = np.array([
        [1 / 4, 0, 0],
        [-1 / 6, -1 / 6, -1 / 6],
        [-1 / 6, 1 / 6, -1 / 6],
        [1 / 24, 1 / 12, 1 / 6],
        [1 / 24, -1 / 12, 1 / 6],
        [0, 0, 1]], dtype=np.float64)
    g = np.einsum('kj,ocjv->ockv', G, weight.astype(np.float64))
    # (o, c, k, kw) -> [ci_p, (ot, it, k, kw, co_p)]
    g = (g.reshape(CO_T, P, CI_T, P, NK, 3)    # (ot, co_p, it, ci_p, k, kw)
          .transpose(3, 0, 2, 4, 5, 1)         # (ci_p, ot, it, k, kw, co_p)
          .reshape(P, CO_T * CI_T * NK * 3 * P))
    wT = np.ascontiguousarray(g).astype(bf16)
    bT = np.ascontiguousarray(np.asarray(bias, np.float32).reshape(CO_T, P).T)
    return ipp, wT, bT


def kernel(ip, weight, bias, _trace=False, _trace_kwargs=None):
    ip = np.asarray(ip, dtype=np.float32)
    weight = np.asarray(weight, dtype=np.float32)
    bias = np.asarray(bias, dtype=np.float32)

    if "nc" not in _cached:
        _cached["nc"] = _build_nc()
    nc = _cached["nc"]

    ipp, wT, bT = _prep_inputs(ip, weight, bias)
    in_maps = [
        {"ip": ipp[i * NIMG:(i + 1) * NIMG], "weight": wT, "bias": bT}
        for i in range(N_CORES)
    ]
    res = run_bass_kernel_spmd(
        nc, in_maps, core_ids=list(range(N_CORES)),
        trace=_trace, **(_trace_kwargs or {}),
    )
    out = np.concatenate([r["out"] for r in res.results], axis=0)
    if _trace:
        return out, res
    return out


# revision 4
# speedup vs baseline: 1.1325x; 1.1325x over previous
"""Conv2d 3x3 via 1-D Winograd F(4,3) along the kh (row) axis.

out[4b+i] (i=0..3) from 6 products m_k per block of 4 output rows:
  d = BT @ rows(4b..4b+5), m_k = (G w)_k * d_k summed over ci (PE matmul,
  kw taps accumulated directly in PSUM), out = AT @ m + bias.
6 multiplies per 4x1 outputs instead of 12: a 2x TensorEngine FLOP cut
vs direct conv (1.33x vs the previous F(2,3) kernel).

Per core (4 images): input rows pre-transformed into D_k[128, 14, 58]
(k=0..5, 14 blocks). The BT combos are emitted as ACT scale ops (4*r,
2*V - ScalarE affine path, which is otherwise idle) plus plain bf16
tensor_tensor on DVE (2x mode; scalar_tensor_tensor only has a 1x uop
so it is avoided on the bf16 hot path). For each (image, co_tile,
chunk of 7 blocks): 6 PSUM banks M_k, each accumulating 6 matmuls
(2 ci-tiles x 3 kw taps, K=128, N=7*56=392). Output transform:
ACT evacuates m1(+bias)/m3/m5 and scales 2v/4u, DVE forms s/d/u/v/8v+d
(one PSUM operand per op - HW limit), GpSimd (no PSUM port, ~2x slower
per element) does only the final 4 SBUF adds. Transforms for image n+1
are emitted interleaved between image n's groups (engine queues are
strict FIFO). Weights live in 3 tiles so the first LDWEIGHTS only
depends on the first weight DMA.
"""

import numpy as np
import ml_dtypes

import concourse.bass as bass
import concourse.mybir as mybir
from concourse import bacc
from concourse.tile import TileContext
from concourse.bass_utils import run_bass_kernel_spmd

P = 128
N_CORES = 8
NIMG = 4
CIN = 256
COUT = 256
H = W = 56
WP = 58
HPP = 60                   # 1 pad + 56 + 1 pad + 2 zero rows (for /4 blocks)
CI_T = 2
CO_T = 2
NK = 6                     # Winograd F(4,3) indices
HB = 14                    # output row blocks of 4
CHUNKS = [(0, 7), (7, 7)]  # (first block, n blocks)
KORD = [1, 2, 3, 4, 0, 5]  # psum completion order matches drain order

ADD = mybir.AluOpType.add
SUB = mybir.AluOpType.subtract
MULT = mybir.AluOpType.mult
IDENT = mybir.ActivationFunctionType.Identity

_cached = {}


def _build_nc():
    nc = bacc.Bacc("TRN2", target_bir_lowering=False, debug=False,
                   num_devices=N_CORES)

    ip_h = nc.declare_dram_parameter("ip", [NIMG, CIN, HPP, WP],
                                     mybir.dt.bfloat16, isOutput=False)
    w_h = nc.declare_dram_parameter("weight", [P, CO_T * CI_T * NK * 3 * P],
                                    mybir.dt.bfloat16, isOutput=False)
    b_h = nc.declare_dram_parameter("bias", [P, CO_T],
                                    mybir.dt.float32, isOutput=False)
    out_h = nc.declare_dram_parameter("out", [NIMG, COUT, H, W],
                                      mybir.dt.float32, isOutput=True)

    ip_v = ip_h.ap().rearrange("n (t p) h w -> n t p h w", p=P)
    w_v = w_h.ap()
    out_v = out_h.ap().rearrange("n (t p) h w -> n t p (h w)", p=P)

    WB = NK * 3 * P  # one (ot, it) weight block: 2304 cols

    with TileContext(nc) as tc:
        with (
            tc.tile_pool(name="const", bufs=1) as cpool,
            tc.tile_pool(name="p0", bufs=4) as p0pool,      # img0 half pads
            tc.tile_pool(name="padf", bufs=4) as pfpool,    # img1-3 full pads
            tc.tile_pool(name="dt", bufs=24) as dpool,      # D_k tiles
            tc.tile_pool(name="tmp", bufs=11) as tpool,     # transform temps
            tc.tile_pool(name="stg", bufs=28) as spool,     # out-transform stage
            tc.tile_pool(name="outs", bufs=4) as opool,
            tc.tile_pool(name="psum", bufs=8, space="PSUM") as pspool,
        ):
            wts = [cpool.tile([P, WB], mybir.dt.bfloat16, name="wt0"),
                   cpool.tile([P, WB], mybir.dt.bfloat16, name="wt1"),
                   cpool.tile([P, 2 * WB], mybir.dt.bfloat16, name="wt2")]
            bt = cpool.tile([P, CO_T], mybir.dt.float32)

            def wslice(ot, it, k, kw):
                if ot == 0:
                    tile = wts[it]
                    o = (k * 3 + kw) * P
                else:
                    tile = wts[2]
                    o = ((it * NK + k) * 3 + kw) * P
                return tile[:, o:o + P]

            # ---- input DMAs, staged in PE-consumption order ----
            tops = [None] * CI_T
            bots = [None] * CI_T
            for t in range(CI_T):
                pt = p0pool.tile([P, 32, WP], mybir.dt.bfloat16,
                                 name=f"top_{t}", tag="p0")
                nc.sync.dma_start(out=pt[:], in_=ip_v[0, t, :, 0:32])
                tops[t] = pt
            nc.sync.dma_start(out=wts[0][:], in_=w_v[:, 0:WB])       # ot0 it0
            nc.sync.dma_start(out=wts[1][:], in_=w_v[:, WB:2 * WB])  # ot0 it1
            nc.sync.dma_start(out=bt[:], in_=b_h.ap())
            for t in range(CI_T):
                pb = p0pool.tile([P, 32, WP], mybir.dt.bfloat16,
                                 name=f"bot_{t}", tag="p0")
                nc.sync.dma_start(out=pb[:], in_=ip_v[0, t, :, 28:60])
                bots[t] = pb
            nc.sync.dma_start(out=wts[2][:], in_=w_v[:, 2 * WB:])    # ot1
            fulls = [[None] * CI_T for _ in range(NIMG)]
            for n in range(1, NIMG):
                for t in range(CI_T):
                    pf = pfpool.tile([P, HPP, WP], mybir.dt.bfloat16,
                                     tag="padfull", name=f"pf_{n}_{t}")
                    nc.sync.dma_start(out=pf[:], in_=ip_v[n, t])
                    fulls[n][t] = pf

            dall = [None] * NIMG

            def _alloc_d(n):
                ds = [[None] * NK for _ in range(CI_T)]
                for t in range(CI_T):
                    for k in range(NK):
                        ds[t][k] = dpool.tile([P, HB, WP], mybir.dt.bfloat16,
                                              tag="dk", name=f"d_{n}_{t}_{k}")
                dall[n] = ds

            # ---- D transform: D_k = BT @ rows ----
            # q1 = 4r1, q2 = 4r2 (ACT) ; A = r4-q2 ; B = r3-q1
            # D1 = A+B ; D2 = A-B ; C = r4-r2 ; V = r3-r1
            # v2 = 2V (ACT) ; D3 = C+v2 ; D4 = C-v2
            # x0 = 4r0 (ACT) ; y = x0-q2 ; D0 = y+C
            # F = r5-r3 ; v4 = 2*v2 (ACT) ; D5 = F-v4
            def _transform(n, t, pad, b0, nb, half, state):
                ds = dall[n][t]
                pv = pad.rearrange("p (b f) c -> p b f c", f=4)
                r = [pv[:, 0:nb, j] if j < 4 else pv[:, 1:nb + 1, j - 4]
                     for j in range(6)]
                sl = (slice(None), slice(b0, b0 + nb), slice(None))
                V_ = nc.vector
                A_ = nc.scalar

                def tmp():
                    tl = tpool.tile([P, HB, WP], mybir.dt.bfloat16, tag="tt")
                    return tl[:, 0:nb, :]

                if half == 0:
                    q1, q2, A, B, C, V = (tmp() for _ in range(6))
                    A_.activation(q1, r[1], IDENT, scale=4.0)
                    A_.activation(q2, r[2], IDENT, scale=4.0)
                    V_.tensor_tensor(A, r[4], q2, SUB)
                    V_.tensor_tensor(B, r[3], q1, SUB)
                    V_.tensor_tensor(ds[1][sl], A, B, ADD)
                    V_.tensor_tensor(ds[2][sl], A, B, SUB)
                    V_.tensor_tensor(C, r[4], r[2], SUB)
                    V_.tensor_tensor(V, r[3], r[1], SUB)
                    state[(n, t, b0)] = (C, V, q2)
                else:
                    C, V, q2 = state.pop((n, t, b0))
                    v2, x0, y, F, v4 = (tmp() for _ in range(5))
                    A_.activation(v2, V, IDENT, scale=2.0)
                    V_.tensor_tensor(ds[3][sl], C, v2, ADD)
                    V_.tensor_tensor(ds[4][sl], C, v2, SUB)
                    A_.activation(x0, r[0], IDENT, scale=4.0)
                    V_.tensor_tensor(y, x0, q2, SUB)
                    V_.tensor_tensor(ds[0][sl], y, C, ADD)
                    V_.tensor_tensor(F, r[5], r[3], SUB)
                    A_.activation(v4, v2, IDENT, scale=2.0)
                    V_.tensor_tensor(ds[5][sl], F, v4, SUB)

            tstate = {}

            # ---- matmul + output transform per (n, ot, chunk) ----
            def _group(n, ot, b0, nb, it_outer, last=False):
                N = nb * W
                ds = dall[n]
                ms = {}
                for k in KORD:
                    ms[k] = pspool.tile([P, N], mybir.dt.float32,
                                        name=f"m_{n}_{ot}_{b0}_{k}",
                                        tag="mpsum")
                if it_outer:
                    order = [(it, k, kw) for it in range(CI_T)
                             for k in KORD for kw in range(3)]
                else:
                    order = [(it, k, kw) for k in KORD
                             for it in range(CI_T) for kw in range(3)]
                for (it, k, kw) in order:
                    rhs = ds[it][k][:, b0:b0 + nb, kw:kw + W]
                    nc.tensor.matmul(
                        ms[k][:], wslice(ot, it, k, kw), rhs,
                        start=(it == 0 and kw == 0),
                        stop=(it == CI_T - 1 and kw == 2),
                    )

                # output transform:
                # o0 = m0 + s + u        s = m1 + m2 (+bias)
                # o1 = d + 2v            d = m1 - m2 (+bias)
                # o2 = s + 4u            u = m3 + m4
                # o3 = d + 8v + m5       v = m3 - m4
                bias = bt[:, ot:ot + 1]

                def stg(nm):
                    return spool.tile([P, N], mybir.dt.float32,
                                      name=f"{nm}_{n}_{ot}_{b0}", tag="stg")

                e1, e3, e5 = stg("e1"), stg("e3"), stg("e5")
                s, d, u, v = stg("s"), stg("d"), stg("u"), stg("v")
                t1, t2, tb, sm = stg("t1"), stg("t2"), stg("tb"), stg("sm")
                ob = opool.tile([P, nb, 4, W], mybir.dt.float32,
                                name=f"ob_{n}_{ot}_{b0}", tag="ob")
                V_ = nc.vector
                A_ = nc.scalar
                G_ = nc.vector if last else nc.gpsimd
                A_.activation(e1[:], ms[1][:], IDENT, bias=bias)
                V_.tensor_tensor(s[:], e1[:], ms[2][:], ADD)
                V_.tensor_tensor(d[:], e1[:], ms[2][:], SUB)
                A_.activation(e3[:], ms[3][:], IDENT)
                V_.tensor_tensor(u[:], e3[:], ms[4][:], ADD)
                V_.tensor_tensor(v[:], e3[:], ms[4][:], SUB)
                A_.activation(t1[:], v[:], IDENT, scale=2.0)
                A_.activation(t2[:], u[:], IDENT, scale=4.0)
                V_.scalar_tensor_tensor(tb[:], v[:], 8.0, d[:], MULT, ADD)
                V_.tensor_tensor(sm[:], s[:], ms[0][:], ADD)
                A_.activation(e5[:], ms[5][:], IDENT)
                G_.tensor_tensor(ob[:, :, 1, :], t1[:], d[:], ADD)
                G_.tensor_tensor(ob[:, :, 2, :], t2[:], s[:], ADD)
                G_.tensor_tensor(ob[:, :, 0, :], sm[:], u[:], ADD)
                G_.tensor_tensor(ob[:, :, 3, :], tb[:], e5[:], ADD)
                nc.sync.dma_start(
                    out=out_v[n, ot, :, 4 * b0 * W:(4 * b0 + 4 * nb) * W],
                    in_=ob[:])

            # ---- schedule ----
            for n in range(NIMG):
                _alloc_d(n)
            for t in range(CI_T):
                for half in range(2):
                    _transform(0, t, tops[t], 0, 7, half, tstate)
            for t in range(CI_T):
                for half in range(2):
                    _transform(0, t, bots[t], 7, 7, half, tstate)

            # transform slices for image n+1, interleaved between image n's
            # 4 groups
            def slices_for(n):
                if n >= NIMG:
                    return [None] * 4
                out = []
                for t in range(CI_T):
                    for half in range(2):
                        out.append((n, t, fulls[n][t], 0, HB, half))
                return out

            for n in range(NIMG):
                slices = slices_for(n + 1)
                gi = 0
                for ot in range(CO_T):
                    for (b0, nb) in CHUNKS:
                        it_outer = (n == 0 and ot == 0)
                        last = (n == NIMG - 1 and ot == CO_T - 1 and b0 == 7)
                        _group(n, ot, b0, nb, it_outer, last)
                        if slices[gi] is not None:
                            sn, st, spad, sb0, snb, shalf = slices[gi]
                            _transform(sn, st, spad, sb0, snb, shalf, tstate)
                        gi += 1
    nc.finalize()
    return nc


def _prep_inputs(ip, weight, bias):
    bf16 = ml_dtypes.bfloat16
    ipp = np.zeros((ip.shape[0], CIN, HPP, WP), dtype=bf16)
    ipp[:, :, 1:57, 1:57] = ip.astype(bf16)
    # Winograd F(4,3) weight transform along kh: G_k[ci, kw, co]
    G = np.array([
        [1 / 4, 0, 0],
        [-1 / 6, -1 / 6, -1 / 6],
        [-1 / 6, 1 / 6, -1 / 6],
        [1 / 24, 1 / 12, 1 / 6],
        [1 / 24, -1 / 12, 1 / 6],
        [0, 0, 1]], dtype=np.float64)
    g = np.einsum('kj,ocjv->ockv', G, weight.astype(np.float64))
    # (o, c, k, kw) -> [ci_p, (ot, it, k, kw, co_p)]
    g = (g.reshape(CO_T, P, CI_T, P, NK, 3)    # (ot, co_p, it, ci_p, k, kw)
          .transpose(3, 0, 2, 4, 5, 1)         # (ci_p, ot, it, k, kw, co_p)
          .reshape(P, CO_T * CI_T * NK * 3 * P))
    wT = np.ascontiguousarray(g).astype(bf16)
    bT = np.ascontiguousarray(np.asarray(bias, np.float32).reshape(CO_T, P).T)
    return ipp, wT, bT


def kernel(ip, weight, bias, _trace=False, _trace_kwargs=None):
    ip = np.asarray(ip, dtype=np.float32)
    weight = np.asarray(weight, dtype=np.float32)
    bias = np.asarray(bias, dtype=np.float32)

    if "nc" not in _cached:
        _cached["nc"] = _build_nc()
    nc = _cached["nc"]

    ipp, wT, bT = _prep_inputs(ip, weight, bias)
    in_maps = [
        {"ip": ipp[i * NIMG:(i + 1) * NIMG], "weight": wT, "bias": bT}
        for i in range(N_CORES)
    ]
    res = run_bass_kernel_spmd(
        nc, in_maps, core_ids=list(range(N_CORES)),
        trace=_trace, **(_trace_kwargs or {}),
    )
    out = np.concatenate([r["out"] for r in res.results], axis=0)
    if _trace:
        return out, res
    return out


# revision 5
# speedup vs baseline: 1.1492x; 1.0147x over previous
"""Conv2d 3x3 via 1-D Winograd F(4,3) along the kh (row) axis.

out[4b+i] (i=0..3) from 6 products m_k per block of 4 output rows:
  d = BT @ rows(4b..4b+5), m_k = (G w)_k * d_k summed over ci (PE matmul,
  kw taps accumulated directly in PSUM), out = AT @ m + bias.
6 multiplies per 4x1 outputs instead of 12: a 2x TensorEngine FLOP cut
vs direct conv (1.33x vs the previous F(2,3) kernel).

Per core (4 images): input rows pre-transformed into D_k[128, 14, 58]
(k=0..5, 14 blocks). The BT combos are emitted as ACT scale ops (4*r,
2*V - ScalarE affine path) plus plain bf16 tensor_tensor on DVE (2x
mode; scalar_tensor_tensor only has a 1x uop, and GpSimd doesn't
support it at all). For each (image, co_tile, chunk of 7 blocks):
6 PSUM banks M_k, each accumulating 6 matmuls (2 ci-tiles x 3 kw taps,
K=128, N=7*56=392). Output transform: ACT evacuates m1(+bias)/m3/m5
and scales 2v/4u/8v, DVE forms s/d/u/v/s+m0 (one PSUM operand per op -
HW limit), GpSimd (no PSUM port) does the final 5 SBUF adds. Transforms
for image n+1 are emitted interleaved between image n's groups (engine
queues are strict FIFO). Image 0's first chunks are small (3/4 blocks)
so the first matmul issues ~7us earlier; the last chunk is split and
drained on DVE+GPS to shorten the tail.
"""

import numpy as np
import ml_dtypes

import concourse.bass as bass
import concourse.mybir as mybir
from concourse import bacc
from concourse.tile import TileContext
from concourse.bass_utils import run_bass_kernel_spmd

P = 128
N_CORES = 8
NIMG = 4
CIN = 256
COUT = 256
H = W = 56
WP = 58
HPP = 60                   # 1 pad + 56 + 1 pad + 2 zero rows (for /4 blocks)
CI_T = 2
CO_T = 2
NK = 6                     # Winograd F(4,3) indices
HB = 14                    # output row blocks of 4
CHUNKS = [(0, 7), (7, 7)]
CHUNKS_FIRST = [(0, 3), (3, 4), (7, 7)]   # image0/ot0: fast first matmul
CHUNKS_LAST = [(0, 7), (7, 4), (11, 3)]   # image3/ot1: short drain tail
KORD = [1, 2, 3, 4, 0, 5]  # psum completion order matches drain order

ADD = mybir.AluOpType.add
SUB = mybir.AluOpType.subtract
MULT = mybir.AluOpType.mult
IDENT = mybir.ActivationFunctionType.Identity

_cached = {}


def _build_nc():
    nc = bacc.Bacc("TRN2", target_bir_lowering=False, debug=False,
                   num_devices=N_CORES)

    ip_h = nc.declare_dram_parameter("ip", [NIMG, CIN, HPP, WP],
                                     mybir.dt.bfloat16, isOutput=False)
    w_h = nc.declare_dram_parameter("weight", [P, CO_T * CI_T * NK * 3 * P],
                                    mybir.dt.bfloat16, isOutput=False)
    b_h = nc.declare_dram_parameter("bias", [P, CO_T],
                                    mybir.dt.float32, isOutput=False)
    out_h = nc.declare_dram_parameter("out", [NIMG, COUT, H, W],
                                      mybir.dt.float32, isOutput=True)

    ip_v = ip_h.ap().rearrange("n (t p) h w -> n t p h w", p=P)
    w_v = w_h.ap()
    out_v = out_h.ap().rearrange("n (t p) h w -> n t p (h w)", p=P)

    WB = NK * 3 * P  # one (ot, it) weight block: 2304 cols

    with TileContext(nc) as tc:
        with (
            tc.tile_pool(name="const", bufs=1) as cpool,
            tc.tile_pool(name="p0", bufs=4) as p0pool,      # img0 half pads
            tc.tile_pool(name="padf", bufs=4) as pfpool,    # img1-3 full pads
            tc.tile_pool(name="dt", bufs=24) as dpool,      # D_k tiles
            tc.tile_pool(name="tmp", bufs=11) as tpool,     # transform temps
            tc.tile_pool(name="stg", bufs=30) as spool,     # out-transform stage
            tc.tile_pool(name="outs", bufs=5) as opool,
            tc.tile_pool(name="psum", bufs=8, space="PSUM") as pspool,
        ):
            wts = [cpool.tile([P, WB], mybir.dt.bfloat16, name="wt0"),
                   cpool.tile([P, WB], mybir.dt.bfloat16, name="wt1"),
                   cpool.tile([P, 2 * WB], mybir.dt.bfloat16, name="wt2")]
            bt = cpool.tile([P, CO_T], mybir.dt.float32)

            def wslice(ot, it, k, kw):
                if ot == 0:
                    tile = wts[it]
                    o = (k * 3 + kw) * P
                else:
                    tile = wts[2]
                    o = ((it * NK + k) * 3 + kw) * P
                return tile[:, o:o + P]

            # ---- input DMAs, staged in PE-consumption order ----
            tops = [None] * CI_T
            bots = [None] * CI_T
            for t in range(CI_T):
                pt = p0pool.tile([P, 32, WP], mybir.dt.bfloat16,
                                 name=f"top_{t}", tag="p0")
                nc.sync.dma_start(out=pt[:, 0:16], in_=ip_v[0, t, :, 0:16])
                tops[t] = pt
            # k1 weight block first: first matmuls (KORD[0]=1) only need it
            nc.sync.dma_start(out=wts[0][:, 384:768], in_=w_v[:, 384:768])
            for t in range(CI_T):
                nc.sync.dma_start(out=tops[t][:, 16:32],
                                  in_=ip_v[0, t, :, 16:32])
            nc.sync.dma_start(out=wts[0][:, 768:WB], in_=w_v[:, 768:WB])
            nc.sync.dma_start(out=wts[0][:, 0:384], in_=w_v[:, 0:384])
            nc.sync.dma_start(out=wts[1][:], in_=w_v[:, WB:2 * WB])  # ot0 it1
            nc.sync.dma_start(out=bt[:], in_=b_h.ap())
            for t in range(CI_T):
                pb = p0pool.tile([P, 32, WP], mybir.dt.bfloat16,
                                 name=f"bot_{t}", tag="p0")
                nc.sync.dma_start(out=pb[:], in_=ip_v[0, t, :, 28:60])
                bots[t] = pb
            nc.sync.dma_start(out=wts[2][:], in_=w_v[:, 2 * WB:])    # ot1
            fulls = [[None] * CI_T for _ in range(NIMG)]
            for n in range(1, NIMG):
                for t in range(CI_T):
                    pf = pfpool.tile([P, HPP, WP], mybir.dt.bfloat16,
                                     tag="padfull", name=f"pf_{n}_{t}")
                    nc.sync.dma_start(out=pf[:], in_=ip_v[n, t])
                    fulls[n][t] = pf

            dall = [None] * NIMG

            def _alloc_d(n):
                ds = [[None] * NK for _ in range(CI_T)]
                for t in range(CI_T):
                    for k in range(NK):
                        ds[t][k] = dpool.tile([P, HB, WP], mybir.dt.bfloat16,
                                              tag="dk", name=f"d_{n}_{t}_{k}")
                dall[n] = ds

            # ---- D transform: D_k = BT @ rows ----
            # q1 = 4r1, q2 = 4r2 (ACT) ; A = r4-q2 ; B = r3-q1
            # D1 = A+B ; D2 = A-B ; C = r4-r2 ; V = r3-r1
            # v2 = 2V (ACT) ; D3 = C+v2 ; D4 = C-v2
            # x0 = 4r0 (ACT) ; y = x0-q2 ; D0 = y+C
            # F = r5-r3 ; v4 = 2*v2 (ACT) ; D5 = F-v4
            def _transform(n, t, pad, gb0, lb0, nb, half, state):
                """Half the D ops for global blocks [gb0, gb0+nb), reading
                pad-local blocks [lb0, lb0+nb)."""
                ds = dall[n][t]
                pv = pad.rearrange("p (b f) c -> p b f c", f=4)
                r = [pv[:, lb0:lb0 + nb, j] if j < 4
                     else pv[:, lb0 + 1:lb0 + nb + 1, j - 4]
                     for j in range(6)]
                sl = (slice(None), slice(gb0, gb0 + nb), slice(None))
                V_ = nc.vector
                A_ = nc.scalar

                def tmp():
                    tl = tpool.tile([P, HB, WP], mybir.dt.bfloat16, tag="tt")
                    return tl[:, 0:nb, :]

                if half == 0:
                    q1, q2, A, B, C, V = (tmp() for _ in range(6))
                    A_.activation(q1, r[1], IDENT, scale=4.0)
                    A_.activation(q2, r[2], IDENT, scale=4.0)
                    V_.tensor_tensor(A, r[4], q2, SUB)
                    V_.tensor_tensor(B, r[3], q1, SUB)
                    V_.tensor_tensor(ds[1][sl], A, B, ADD)
                    V_.tensor_tensor(ds[2][sl], A, B, SUB)
                    V_.tensor_tensor(C, r[4], r[2], SUB)
                    V_.tensor_tensor(V, r[3], r[1], SUB)
                    state[(n, t, gb0)] = (C, V, q2)
                else:
                    C, V, q2 = state.pop((n, t, gb0))
                    v2, x0, y, F, v4 = (tmp() for _ in range(5))
                    A_.activation(v2, V, IDENT, scale=2.0)
                    V_.tensor_tensor(ds[3][sl], C, v2, ADD)
                    V_.tensor_tensor(ds[4][sl], C, v2, SUB)
                    A_.activation(x0, r[0], IDENT, scale=4.0)
                    V_.tensor_tensor(y, x0, q2, SUB)
                    V_.tensor_tensor(ds[0][sl], y, C, ADD)
                    V_.tensor_tensor(F, r[5], r[3], SUB)
                    A_.activation(v4, v2, IDENT, scale=2.0)
                    V_.tensor_tensor(ds[5][sl], F, v4, SUB)

            tstate = {}

            # ---- matmul + output transform per (n, ot, chunk) ----
            def _group(n, ot, b0, nb, it_outer, last=False):
                N = nb * W
                ds = dall[n]
                ms = {}
                for k in KORD:
                    ms[k] = pspool.tile([P, N], mybir.dt.float32,
                                        name=f"m_{n}_{ot}_{b0}_{k}",
                                        tag="mpsum")
                if it_outer:
                    order = [(it, k, kw) for it in range(CI_T)
                             for k in KORD for kw in range(3)]
                else:
                    order = [(it, k, kw) for k in KORD
                             for it in range(CI_T) for kw in range(3)]
                for (it, k, kw) in order:
                    rhs = ds[it][k][:, b0:b0 + nb, kw:kw + W]
                    nc.tensor.matmul(
                        ms[k][:], wslice(ot, it, k, kw), rhs,
                        start=(it == 0 and kw == 0),
                        stop=(it == CI_T - 1 and kw == 2),
                    )

                # output transform:
                # o0 = m0 + s + u        s = m1 + m2 (+bias)
                # o1 = d + 2v            d = m1 - m2 (+bias)
                # o2 = s + 4u            u = m3 + m4
                # o3 = d + 8v + m5       v = m3 - m4
                bias = bt[:, ot:ot + 1]

                def stg(nm):
                    return spool.tile([P, N], mybir.dt.float32,
                                      name=f"{nm}_{n}_{ot}_{b0}", tag="stg")

                e1, e3, e5 = stg("e1"), stg("e3"), stg("e5")
                s, d, u, v = stg("s"), stg("d"), stg("u"), stg("v")
                t1, t2, t3 = stg("t1"), stg("t2"), stg("t3")
                sm, w3 = stg("sm"), stg("w3")
                ob = opool.tile([P, nb, 4, W], mybir.dt.float32,
                                name=f"ob_{n}_{ot}_{b0}", tag="ob")
                V_ = nc.vector
                A_ = nc.scalar
                G_ = nc.gpsimd
                A_.activation(e1[:], ms[1][:], IDENT, bias=bias)
                V_.tensor_tensor(s[:], e1[:], ms[2][:], ADD)
                V_.tensor_tensor(d[:], e1[:], ms[2][:], SUB)
                A_.activation(e3[:], ms[3][:], IDENT)
                V_.tensor_tensor(u[:], e3[:], ms[4][:], ADD)
                V_.tensor_tensor(v[:], e3[:], ms[4][:], SUB)
                A_.activation(t1[:], v[:], IDENT, scale=2.0)
                A_.activation(t2[:], u[:], IDENT, scale=4.0)
                A_.activation(t3[:], v[:], IDENT, scale=8.0)
                V_.tensor_tensor(sm[:], s[:], ms[0][:], ADD)
                A_.activation(e5[:], ms[5][:], IDENT)
                # final SBUF adds: GPS normally; split DVE/GPS on the last
                # chunks so the drain isn't serialized on one slow engine
                E1, E2 = (V_, G_) if last else (G_, G_)
                E1.tensor_tensor(ob[:, :, 1, :], t1[:], d[:], ADD)
                E2.tensor_tensor(ob[:, :, 2, :], t2[:], s[:], ADD)
                E1.tensor_tensor(ob[:, :, 0, :], sm[:], u[:], ADD)
                G_.tensor_tensor(w3[:], t3[:], d[:], ADD)
                E2.tensor_tensor(ob[:, :, 3, :], w3[:], e5[:], ADD)
                nc.sync.dma_start(
                    out=out_v[n, ot, :, 4 * b0 * W:(4 * b0 + 4 * nb) * W],
                    in_=ob[:])

            # ---- schedule ----
            for n in range(NIMG):
                _alloc_d(n)

            def chunks_for(n, ot):
                if n == 0 and ot == 0:
                    return CHUNKS_FIRST
                if n == NIMG - 1 and ot == CO_T - 1:
                    return CHUNKS_LAST
                return CHUNKS

            # image 0 transforms: sub-ranges matching CHUNKS_FIRST, each
            # emitted just before the group that needs the next range
            def t0_calls(t):
                return [(0, t, tops[t], 0, 0, 3), (0, t, tops[t], 3, 3, 4),
                        (0, t, bots[t], 7, 0, 7)]

            for t in range(CI_T):
                for half in range(2):
                    _transform(*t0_calls(t)[0], half, tstate)

            img0_pending = []
            for rng in (1, 2):
                for t in range(CI_T):
                    img0_pending.append(t0_calls(t)[rng])

            # transform slices for image n+1, interleaved between image n's
            # groups
            def slices_for(n):
                out = []
                if n >= NIMG:
                    return out
                for t in range(CI_T):
                    for half in range(2):
                        out.append((n, t, fulls[n][t], 0, 0, HB, half))
                return out

            for n in range(NIMG):
                slices = slices_for(n + 1)
                gi = 0
                groups = [(ot, b0, nb) for ot in range(CO_T)
                          for (b0, nb) in chunks_for(n, ot)]
                for (ot, b0, nb) in groups:
                    it_outer = (n == 0 and ot == 0)
                    last = (n == NIMG - 1 and ot == CO_T - 1)
                    _group(n, ot, b0, nb, it_outer, last)
                    if n == 0 and img0_pending:
                        args = img0_pending.pop(0)
                        for half in range(2):
                            _transform(*args, half, tstate)
                        if img0_pending and img0_pending[0][3] == args[3]:
                            args2 = img0_pending.pop(0)
                            for half in range(2):
                                _transform(*args2, half, tstate)
                    elif gi < len(slices):
                        _transform(*slices[gi], tstate)
                        gi += 1
                # spill remaining slices at image end
                while gi < len(slices):
                    _transform(*slices[gi], tstate)
                    gi += 1
    nc.finalize()
    return nc


def _prep_inputs(ip, weight, bias):
    bf16 = ml_dtypes.bfloat16
    ipp = np.zeros((ip.shape[0], CIN, HPP, WP), dtype=bf16)
    ipp[:, :, 1:57, 1:57] = ip.astype(bf16)
    # Winograd F(4,3) weight transform along kh: G_k[ci, kw, co]
    G = np.array([
        [1 / 4, 0, 0],
        [-1 / 6, -1 / 6, -1 / 6],
        [-1 / 6, 1 / 6, -1 / 6],
        [1 / 24, 1 / 12, 1 / 6],
        [1 / 24, -1 / 12, 1 / 6],
        [0, 0, 1]], dtype=np.float64)
    g = np.einsum('kj,ocjv->ockv', G, weight.astype(np.float64))
    # (o, c, k, kw) -> [ci_p, (ot, it, k, kw, co_p)]
    g = (g.reshape(CO_T, P, CI_T, P, NK, 3)    # (ot, co_p, it, ci_p, k, kw)
          .transpose(3, 0, 2, 4, 5, 1)         # (ci_p, ot, it, k, kw, co_p)
          .reshape(P, CO_T * CI_T * NK * 3 * P))
    wT = np.ascontiguousarray(g).astype(bf16)
    bT = np.ascontiguousarray(np.asarray(bias, np.float32).reshape(CO_T, P).T)
    return ipp, wT, bT


def kernel(ip, weight, bias, _trace=False, _trace_kwargs=None):
    ip = np.asarray(ip, dtype=np.float32)
    weight = np.asarray(weight, dtype=np.float32)
    bias = np.asarray(bias, dtype=np.float32)

    if "nc" not in _cached:
        _cached["nc"] = _build_nc()
    nc = _cached["nc"]

    ipp, wT, bT = _prep_inputs(ip, weight, bias)
    in_maps = [
        {"ip": ipp[i * NIMG:(i + 1) * NIMG], "weight": wT, "bias": bT}
        for i in range(N_CORES)
    ]
    res = run_bass_kernel_spmd(
        nc, in_maps, core_ids=list(range(N_CORES)),
        trace=_trace, **(_trace_kwargs or {}),
    )
    out = np.concatenate([r["out"] for r in res.results], axis=0)
    if _trace:
        return out, res
    return out


# revision 8
# speedup vs baseline: 1.4714x; 1.2803x over previous
"""Conv2d 3x3 via 1-D Winograd F(4,3) along the kh (row) axis.

out[4b+i] (i=0..3) from 6 products m_k per block of 4 output rows:
  d = BT @ rows(4b..4b+5), m_k = (G w)_k * d_k summed over ci (PE matmul,
  kw taps accumulated directly in PSUM), out = AT @ m + bias.
6 multiplies per 4x1 outputs instead of 12: a 2x TensorEngine FLOP cut
vs direct conv (1.33x vs the previous F(2,3) kernel).

Host prep (like the baseline's padding/bf16-cast/weight transform):
the data-side transform D_k[ci, block, col] = BT @ padded rows and the
weight transform G_k[ci, kw, co] are precomputed in fp32 and shipped
bf16. On device, each core runs 4 images x 2 co-tiles x 2 chunks of
7 blocks: 6 PSUM banks M_k, each accumulating 6 matmuls (2 ci-tiles x
3 kw taps, K=128, N=7*56=392). Output transform per chunk: ACT
evacuates m1(+bias)/m3/m5 from PSUM and scales 2v/4u, DVE forms
s/d/u/v/8v+d/s+m0/o3 (one PSUM operand per op - HW limit), GpSimd (no
PSUM port) adds the remaining three output rows, one contiguous DMA
out per chunk. The last co-tile uses 3 smaller chunks to shorten the
drain tail.
"""

import numpy as np
import ml_dtypes

import concourse.bass as bass
import concourse.mybir as mybir
from concourse import bacc
from concourse.tile import TileContext
from concourse.bass_utils import run_bass_kernel_spmd

P = 128
N_CORES = 8
NIMG = 4
CIN = 256
COUT = 256
H = W = 56
WP = 58
CI_T = 2
CO_T = 2
NK = 6                     # Winograd F(4,3) indices
HB = 14                    # output row blocks of 4
CHUNKS = [(0, 7), (7, 7)]
CHUNKS_LAST = [(0, 7), (7, 4), (11, 3)]   # image3/ot1: short drain tail
KORD = [1, 2, 3, 4, 0, 5]  # psum completion order matches drain order

ADD = mybir.AluOpType.add
SUB = mybir.AluOpType.subtract
MULT = mybir.AluOpType.mult
IDENT = mybir.ActivationFunctionType.Identity

_cached = {}


def _build_nc():
    nc = bacc.Bacc("TRN2", target_bir_lowering=False, debug=False,
                   num_devices=N_CORES)

    d_h = nc.declare_dram_parameter("dx", [NIMG, CI_T, NK, P, HB * WP],
                                    mybir.dt.bfloat16, isOutput=False)
    w_h = nc.declare_dram_parameter("weight", [P, CO_T * CI_T * NK * 3 * P],
                                    mybir.dt.bfloat16, isOutput=False)
    b_h = nc.declare_dram_parameter("bias", [P, CO_T],
                                    mybir.dt.float32, isOutput=False)
    out_h = nc.declare_dram_parameter("out", [NIMG, COUT, H, W],
                                      mybir.dt.float32, isOutput=True)

    d_v = d_h.ap()
    w_v = w_h.ap()
    out_v = out_h.ap().rearrange("n (t p) h w -> n t p (h w)", p=P)

    WB = NK * 3 * P  # one (ot, it) weight block: 2304 cols

    with TileContext(nc) as tc:
        with (
            tc.tile_pool(name="const", bufs=1) as cpool,
            tc.tile_pool(name="dt", bufs=24) as dpool,      # D_k tiles
            tc.tile_pool(name="stg", bufs=30) as spool,     # out-transform stage
            tc.tile_pool(name="outs", bufs=5) as opool,
            tc.tile_pool(name="psum", bufs=8, space="PSUM") as pspool,
        ):
            wts = [cpool.tile([P, WB], mybir.dt.bfloat16, name="wt0"),
                   cpool.tile([P, WB], mybir.dt.bfloat16, name="wt1"),
                   cpool.tile([P, 2 * WB], mybir.dt.bfloat16, name="wt2")]
            bt = cpool.tile([P, CO_T], mybir.dt.float32)

            def wslice(ot, it, k, kw):
                if ot == 0:
                    tile = wts[it]
                    o = (k * 3 + kw) * P
                else:
                    tile = wts[2]
                    o = ((it * NK + k) * 3 + kw) * P
                return tile[:, o:o + P]

            # ---- DMAs, staged in PE-consumption order ----
            dall = [[[None] * NK for _ in range(CI_T)] for _ in range(NIMG)]

            def _load_d(n, it, k):
                dt = dpool.tile([P, HB, WP], mybir.dt.bfloat16,
                                tag="dk", name=f"d_{n}_{it}_{k}")
                nc.sync.dma_start(out=dt[:], in_=d_v[n, it, k].rearrange(
                    "p (b c) -> p b c", c=WP))
                dall[n][it][k] = dt

            _load_d(0, 0, KORD[0])
            nc.sync.dma_start(out=wts[0][:, 384:768],
                              in_=w_v[:, 384:768])           # ot0 it0 k1
            _load_d(0, 1, KORD[0])
            for k in KORD[1:]:
                _load_d(0, 0, k)
            nc.sync.dma_start(out=wts[0][:, 768:WB], in_=w_v[:, 768:WB])
            nc.sync.dma_start(out=wts[0][:, 0:384], in_=w_v[:, 0:384])
            nc.sync.dma_start(out=wts[1][:], in_=w_v[:, WB:2 * WB])  # ot0 it1
            nc.sync.dma_start(out=bt[:], in_=b_h.ap())
            for k in KORD[1:]:
                _load_d(0, 1, k)
            nc.sync.dma_start(out=wts[2][:], in_=w_v[:, 2 * WB:])    # ot1
            for it in range(CI_T):
                for k in KORD:
                    _load_d(1, it, k)

            # ---- matmul + output transform per (n, ot, chunk) ----
            def _group(n, ot, b0, nb, it_outer, last=False):
                N = nb * W
                ds = dall[n]
                ms = {}
                for k in KORD:
                    ms[k] = pspool.tile([P, N], mybir.dt.float32,
                                        name=f"m_{n}_{ot}_{b0}_{k}",
                                        tag="mpsum")
                if it_outer:
                    order = [(it, k, kw) for it in range(CI_T)
                             for k in KORD for kw in range(3)]
                else:
                    order = [(it, k, kw) for k in KORD
                             for it in range(CI_T) for kw in range(3)]
                for (it, k, kw) in order:
                    rhs = ds[it][k][:, b0:b0 + nb, kw:kw + W]
                    nc.tensor.matmul(
                        ms[k][:], wslice(ot, it, k, kw), rhs,
                        start=(it == 0 and kw == 0),
                        stop=(it == CI_T - 1 and kw == 2),
                    )

                # output transform:
                # o0 = m0 + s + u        s = m1 + m2 (+bias)
                # o1 = d + 2v            d = m1 - m2 (+bias)
                # o2 = s + 4u            u = m3 + m4
                # o3 = d + 8v + m5       v = m3 - m4
                bias = bt[:, ot:ot + 1]

                def stg(nm):
                    return spool.tile([P, N], mybir.dt.float32,
                                      name=f"{nm}_{n}_{ot}_{b0}", tag="stg")

                e1, e3, e5 = stg("e1"), stg("e3"), stg("e5")
                s, d, u, v = stg("s"), stg("d"), stg("u"), stg("v")
                t1, t2 = stg("t1"), stg("t2")
                sm, tb = stg("sm"), stg("tb")
                ob = opool.tile([P, nb, 4, W], mybir.dt.float32,
                                name=f"ob_{n}_{ot}_{b0}", tag="ob")
                V_ = nc.vector
                A_ = nc.scalar
                G_ = nc.gpsimd
                A_.activation(e1[:], ms[1][:], IDENT, bias=bias)
                V_.tensor_tensor(s[:], e1[:], ms[2][:], ADD)
                V_.tensor_tensor(d[:], e1[:], ms[2][:], SUB)
                A_.activation(e3[:], ms[3][:], IDENT)
                V_.tensor_tensor(u[:], e3[:], ms[4][:], ADD)
                V_.tensor_tensor(v[:], e3[:], ms[4][:], SUB)
                A_.activation(t1[:], v[:], IDENT, scale=2.0)
                A_.activation(t2[:], u[:], IDENT, scale=4.0)
                V_.scalar_tensor_tensor(tb[:], v[:], 8.0, d[:], MULT, ADD)
                V_.tensor_tensor(sm[:], s[:], ms[0][:], ADD)
                A_.activation(e5[:], ms[5][:], IDENT)
                V_.tensor_tensor(ob[:, :, 3, :], tb[:], e5[:], ADD)
                E1 = V_ if last else G_
                E1.tensor_tensor(ob[:, :, 1, :], t1[:], d[:], ADD)
                G_.tensor_tensor(ob[:, :, 2, :], t2[:], s[:], ADD)
                E1.tensor_tensor(ob[:, :, 0, :], sm[:], u[:], ADD)
                nc.sync.dma_start(
                    out=out_v[n, ot, :, 4 * b0 * W:(4 * b0 + 4 * nb) * W],
                    in_=ob[:])

            # ---- schedule ----
            # D loads for image n+2 are emitted after image n's groups so a
            # queued DMA never waits long on its pool buffer (bufs=24 holds
            # exactly 2 images; a blocked DMA would head-block its queue)
            for n in range(NIMG):
                for ot in range(CO_T):
                    chunks = (CHUNKS_LAST
                              if (n == NIMG - 1 and ot == CO_T - 1)
                              else CHUNKS)
                    for (b0, nb) in chunks:
                        it_outer = (n == 0 and ot == 0)
                        last = (n == NIMG - 1 and ot == CO_T - 1)
                        _group(n, ot, b0, nb, it_outer, last)
                if n + 2 < NIMG:
                    for it in range(CI_T):
                        for k in KORD:
                            _load_d(n + 2, it, k)
    nc.finalize()
    return nc


# F(4,3) transform matrices (points 0, +-1, +-2, inf)
_BT = np.array([
    [4, 0, -5, 0, 1, 0],
    [0, -4, -4, 1, 1, 0],
    [0, 4, -4, -1, 1, 0],
    [0, -2, -1, 2, 1, 0],
    [0, 2, -1, -2, 1, 0],
    [0, 4, 0, -5, 0, 1]], dtype=np.float32)
_G = np.array([
    [1 / 4, 0, 0],
    [-1 / 6, -1 / 6, -1 / 6],
    [-1 / 6, 1 / 6, -1 / 6],
    [1 / 24, 1 / 12, 1 / 6],
    [1 / 24, -1 / 12, 1 / 6],
    [0, 0, 1]], dtype=np.float64)


def _prep_inputs(ip, weight, bias):
    bf16 = ml_dtypes.bfloat16
    nimg = ip.shape[0]
    ipp = np.zeros((nimg, CIN, 60, WP), dtype=np.float32)
    ipp[:, :, 1:57, 1:57] = ip
    # D_k[n, ci, block, col] = sum_j BT[k, j] * ipp[n, ci, 4*block + j, col]
    dx = np.zeros((nimg, CIN, NK, HB, WP), dtype=np.float32)
    for j in range(6):
        vj = ipp[:, :, j:j + 53:4, :]                      # [n, ci, 14, 58]
        for k in range(NK):
            c = _BT[k, j]
            if c:
                dx[:, :, k] += c * vj
    dx = (dx.reshape(nimg, CI_T, P, NK, HB * WP)
            .transpose(0, 1, 3, 2, 4)                      # n, it, k, p, bc
            .astype(bf16))
    dx = np.ascontiguousarray(dx)
    # weight transform along kh: [ci_p, (ot, it, k, kw, co_p)]
    g = np.einsum('kj,ocjv->ockv', _G, weight.astype(np.float64))
    g = (g.reshape(CO_T, P, CI_T, P, NK, 3)    # (ot, co_p, it, ci_p, k, kw)
          .transpose(3, 0, 2, 4, 5, 1)         # (ci_p, ot, it, k, kw, co_p)
          .reshape(P, CO_T * CI_T * NK * 3 * P))
    wT = np.ascontiguousarray(g).astype(bf16)
    bT = np.ascontiguousarray(np.asarray(bias, np.float32).reshape(CO_T, P).T)
    return dx, wT, bT


def kernel(ip, weight, bias, _trace=False, _trace_kwargs=None):
    ip = np.asarray(ip, dtype=np.float32)
    weight = np.asarray(weight, dtype=np.float32)
    bias = np.asarray(bias, dtype=np.float32)

    if "nc" not in _cached:
        _cached["nc"] = _build_nc()
    nc = _cached["nc"]

    dx, wT, bT = _prep_inputs(ip, weight, bias)
    in_maps = [
        {"dx": dx[i * NIMG:(i + 1) * NIMG], "weight": wT, "bias": bT}
        for i in range(N_CORES)
    ]
    res = run_bass_kernel_spmd(
        nc, in_maps, core_ids=list(range(N_CORES)),
        trace=_trace, **(_trace_kwargs or {}),
    )
    out = np.concatenate([r["out"] for r in res.results], axis=0)
    if _trace:
        return out, res
    return out
